# revision 25
# baseline (speedup 1.0000x reference)
"""Trainium2 Bass kernel for a single-step GRU attention decoder.

Math (matches the reference nn.Module):
    xe  = emb[x]                                   # [H]
    a   = log_softmax(cat(xe, h0) @ attn_W.T + attn_b)   # [L]
    ap  = a @ encoder_outputs                      # [H]
    g   = relu(cat(xe, ap) @ ctx_W.T + ctx_b)      # [H]
    GRU(g, h0) -> h_new                            # [H]
    logits = h_new @ out_W.T + out_b               # [V]

Distribution across 8 NeuronCores (one TRN2 chip):
  - attention sharded over L (512 rows/core); exploiting linearity,
    log_softmax @ enc == a @ enc - (log sum exp a) * colsum(enc), so one
    AllGather of per-core partials {a@enc, colsum(enc), sum(exp a)} +
    a local tree-sum lets every core reconstruct attn_applied locally.
  - ctx projection sharded by output rows; GRU mats sharded by *input*
    columns so an AllGather+sum of partial (gi, gh) lets every core
    compute the full gates / h_new locally.
  - out projection sharded over vocab (6283 rows/core, padded to 6400).

Perf notes (from perfetto trace analysis of the previous version):
  - every dma_start costs ~610ns of engine issue time and queues transfer
    in issue order -> few, large DMAs issued in consumption-priority
    order (attention weights first, out_W last, vocab-tile-major so the
    final matvec can chase the DMA stream).
  - any NEFF with collectives pays a ~45-70us runtime barrier before the
    first one starts (uncontrollable; dominates the remaining runtime);
    collective staging DMAs ride the scalar engine's HWDGE ring (gpsimd's
    SWDGE ring has a ~29us cold-start) with warmup DMAs absorbing the
    ring's first-use and first-HBM-write latencies.
  - the GRU matvecs run "flipped" (vector stationary, weight matrix
    moving) which turns 96 LDWEIGHTS+matmul pairs into 18 wide matmuls.
  - the PE HAM clock-gate re-throttles to 1.2GHz after ~3.4us idle; junk
    heartbeat matmuls keyed on arriving out_W DMA chunks keep it at
    2.4GHz across the collective waits.

Precision (validated in fp64 simulation against the oracle inputs):
  enc/a single bf16, ctx hi/lo bf16, W_ih hi/lo bf16, whh/out_W single
  bf16, fp32 PSUM accumulation everywhere -> rel err ~6.6e-3 (gate 2e-2).
"""

import ml_dtypes
import numpy as np

import concourse.bass as bass
import concourse.bacc as bacc
import concourse.tile as tile
from concourse import mybir
from concourse.bass_utils import run_bass_kernel_spmd

H = 1024
V = 50257
L = 4096
NCORES = 8
LC = L // NCORES          # 512 encoder rows per core
HC = H // NCORES          # 128 hidden chunk per core
VC = -(-V // NCORES)      # 6283 vocab rows per core
VT = 50                   # vocab tiles of 128 per core
VPAD = VT * 128           # 6400
F32 = mybir.dt.float32
BF16 = mybir.dt.bfloat16
NPBF16 = ml_dtypes.bfloat16
RG = [list(range(NCORES))]

# small_f32 column offsets
S_AB, S_CB, S_H0, S_BI, S_BH, S_OB = 0, 4, 5, 13, 37, 61
S_N = 111
# early_bf16 column offsets
E_CAT, E_ATT, E_ENC = 0, 16, 16 + 16 * LC
E_N = E_ENC + 4 * H       # 12304
# midg_bf16 column offsets
G_WIH, G_WIL, G_WHH, G_H0C = 0, 3 * H, 6 * H, 9 * H
G_N = 9 * H + 1
# out_W chunking (cols of the [128, VT*8*128] tile)
OW_N = VT * 8 * 128       # 51200
OW_CHUNK = 4096           # 4 vocab tiles per DMA chunk
OW_NCHUNK = -(-OW_N // OW_CHUNK)   # 13

_CACHE = {}


def _build(dbg=False):
    key = ("nc", dbg)
    if key in _CACHE:
        return _CACHE[key]

    nc = bacc.Bacc("TRN2", target_bir_lowering=False, debug=False,
                   num_devices=NCORES)

    small_d = nc.dram_tensor("small", [128, S_N], F32, kind="ExternalInput")
    early_d = nc.dram_tensor("early", [128, E_N], BF16, kind="ExternalInput")
    midc_d = nc.dram_tensor("midc", [128, 3 * H], BF16, kind="ExternalInput")
    midg_d = nc.dram_tensor("midg", [128, G_N], BF16, kind="ExternalInput")
    outw_d = nc.dram_tensor("outw", [128, OW_N], BF16, kind="ExternalInput")
    out_d = nc.dram_tensor("out", [128, VT], F32, kind="ExternalOutput")

    AG = "AllGather"
    BYP = mybir.AluOpType.bypass
    ACTF = mybir.ActivationFunctionType

    with tile.TileContext(nc) as tc:
        with (
            tc.tile_pool(name="wp", bufs=1) as wp,
            tc.tile_pool(name="sp", bufs=1) as sp,
            tc.tile_pool(name="pp", bufs=1, space="PSUM") as pp,
            tc.tile_pool(name="dp", bufs=1, space="DRAM") as dp,
        ):
            # ---------------- input DMAs (issue order = priority) --------
            early = wp.tile([128, E_N], BF16, tag="early")
            small = sp.tile([128, S_N], F32, tag="small")
            midc = wp.tile([128, 3 * H], BF16, tag="midc")
            midg = wp.tile([128, G_N], BF16, tag="midg")
            outw = wp.tile([128, OW_N], BF16, tag="outw")

            nc.sync.dma_start(early[:, E_CAT:E_ATT], early_d[:, E_CAT:E_ATT])
            nc.sync.dma_start(small[:], small_d[:])
            half = E_ATT + 8 * LC
            nc.sync.dma_start(early[:, E_ATT:half], early_d[:, E_ATT:half])
            nc.sync.dma_start(early[:, half:E_ENC], early_d[:, half:E_ENC])
            nc.sync.dma_start(early[:, E_ENC:E_N], early_d[:, E_ENC:E_N])
            nc.sync.dma_start(midc[:], midc_d[:])
            nc.sync.dma_start(midg[:], midg_d[:])
            for cth in range(OW_NCHUNK):
                c0, c1 = OW_CHUNK * cth, min(OW_CHUNK * (cth + 1), OW_N)
                nc.sync.dma_start(outw[:, c0:c1], outw_d[:, c0:c1])

            # warm up the scalar engine's HWDGE DMA ring (first use costs
            # ~9us; all collective staging DMAs ride this ring). The second
            # warmup matches cc1_in's exact shape/direction (128 small rows
            # to HBM) — the first HBM-write on a cold ring costs ~10us.
            scr_d = dp.tile([1, 8], F32, tag="scrd")
            nc.scalar.dma_start(scr_d[:], small[0:1, 0:8])
            scr2_d = dp.tile([128, 17], F32, tag="scr2d")
            nc.scalar.dma_start(scr2_d[:], small[:, 0:17])

            # ---------------- attention logits ---------------------------
            # a[l] for the 512 local l as [128, 4] col-major (a[128j+p]).
            # The first 20 matmuls are junk warmup shots (dep: catin DMA
            # only) that pull the PE HAM clock-gate to 2.4GHz before the
            # real attention matmuls arrive; their groups are reset by the
            # real accumulation's start=True.
            a_ps = pp.tile([128, 4], F32, tag="apsA")
            for k in range(20):
                nc.tensor.matmul(a_ps[0:16, k % 4:k % 4 + 1],
                                 early[:, 0:16], early[:, k % 16:k % 16 + 1],
                                 start=True, stop=True)
            for j in range(4):           # l tiles
                for i in range(16):      # cat chunks
                    nc.tensor.matmul(
                        a_ps[:, j:j + 1],
                        early[:, E_ATT + (j * 16 + i) * 128:
                              E_ATT + (j * 16 + i + 1) * 128],
                        early[:, i:i + 1],
                        start=(i == 0), stop=(i == 15))

            # ones_r doubles as the Ln-table preload operand
            ones_r = sp.tile([1, 128], F32, tag="ones_r")
            nc.vector.memset(ones_r[:], 1.0)

            a_sb = sp.tile([128, 4], F32, tag="a")
            nc.vector.tensor_add(a_sb[:], a_ps[:], small[:, S_AB:S_AB + 4])
            exp_sb = sp.tile([128, 4], F32, tag="expa")
            nc.scalar.activation(exp_sb[:], a_sb[:], ACTF.Exp)
            # preload the Ln activation table while CC1 is in flight so the
            # real Ln after the collective skips the ~1.3us ACT_TABLE_LOAD.
            # The input must depend on exp_sb: Tile schedules by deps, and a
            # free-floating warm op gets hoisted BEFORE Exp, whose table
            # load would evict Ln's again.
            lnwarm = sp.tile([1, 1], F32, tag="lnwarm")
            nc.scalar.activation(lnwarm[:], exp_sb[0:1, 0:1], ACTF.Ln)
            a_hi = sp.tile([128, 4], BF16, tag="ahi")
            nc.vector.tensor_copy(a_hi[:], a_sb[:])
            # rhs2: col 2j = a_hi_j, col 2j+1 = 1
            rhs2 = sp.tile([128, 8], BF16, tag="rhs2")
            nc.vector.memset(rhs2[:, 1:8:2], 1.0)
            nc.vector.tensor_copy(rhs2[:, 0:8:2], a_hi[:])

            # sum over partitions+cols of exp(a) -> s
            exp_r = sp.tile([128, 1], F32, tag="expr")
            nc.vector.reduce_sum(exp_r[:], exp_sb[:], axis=mybir.AxisListType.X)
            s_sb = sp.tile([1, 1], F32, tag="s")
            nc.gpsimd.tensor_reduce(s_sb[:], exp_r[:],
                                    axis=mybir.AxisListType.C,
                                    op=mybir.AluOpType.add)

            # enc partials: per h-chunk c, cols (2c, 2c+1) = (e@a, e@1).
            # Split l-tiles (0,1)/(2,3) across two PSUM banks so compute
            # starts before the whole enc DMA lands.
            packA = pp.tile([128, 16], F32, tag="packA")
            for c in range(8):
                for j in range(2):
                    nc.tensor.matmul(
                        packA[:, 2 * c:2 * c + 2],
                        early[:, E_ENC + H * j + 128 * c:
                              E_ENC + H * j + 128 * (c + 1)],
                        rhs2[:, 2 * j:2 * j + 2],
                        start=(j == 0), stop=(j == 1))
            packB = pp.tile([128, 16], F32, tag="packB")
            for c in range(8):
                for j in range(2, 4):
                    nc.tensor.matmul(
                        packB[:, 2 * c:2 * c + 2],
                        early[:, E_ENC + H * j + 128 * c:
                              E_ENC + H * j + 128 * (c + 1)],
                        rhs2[:, 2 * j:2 * j + 2],
                        start=(j == 2), stop=(j == 3))

            # pack: cols 0-7 partial1 per chunk, 8-15 colsum, 16 = s
            # (DVE ops may read at most one PSUM operand, unstrided ->
            # copy to SBUF first, stride there)
            pkA = sp.tile([128, 16], F32, tag="pkA")
            nc.vector.tensor_copy(pkA[:], packA[:])
            pkB = sp.tile([128, 16], F32, tag="pkB")
            nc.vector.tensor_copy(pkB[:], packB[:])
            pack_sb = sp.tile([128, 17], F32, tag="pack")
            nc.vector.memset(pack_sb[:, 16:17], 0.0)
            nc.vector.tensor_add(pack_sb[:, 0:8], pkA[:, 0:16:2],
                                 pkB[:, 0:16:2])
            nc.vector.tensor_add(pack_sb[:, 8:16], pkA[:, 1:16:2],
                                 pkB[:, 1:16:2])
            nc.vector.tensor_copy(pack_sb[0:1, 16:17], s_sb[:])

            # ---------------- collective 1: AllGather + local sum --------
            # the in/out staging DMAs ride the scalar engine's HWDGE ring;
            # gpsimd dma_start uses SWDGE which has a ~29us cold-start.
            cc1_in = dp.tile([128, 17], F32, tag="cc1in")
            cc1_out = dp.tile([NCORES, 128, 17], F32, tag="cc1out",
                              addr_space="Shared")
            nc.scalar.dma_start(cc1_in[:], pack_sb[:])
            nc.gpsimd.collective_compute(AG, BYP, replica_groups=RG,
                                         ins=[cc1_in.opt()],
                                         outs=[cc1_out[:]])
            sums_all = sp.tile([128, 8 * 17], F32, tag="sumsall")
            nc.scalar.dma_start(
                sums_all[:].rearrange("p (r k) -> p r k", r=8),
                cc1_out[:].transpose([1, 0, 2]))
            t1 = sp.tile([128, 68], F32, tag="cc1t1")
            nc.vector.tensor_add(t1[:], sums_all[:, 0:68], sums_all[:, 68:136])
            t2 = sp.tile([128, 34], F32, tag="cc1t2")
            nc.vector.tensor_add(t2[:], t1[:, 0:34], t1[:, 34:68])
            sums = sp.tile([128, 17], F32, tag="sums")
            nc.vector.tensor_add(sums[:], t2[:, 0:17], t2[:, 17:34])

            # c = log(sum exp), broadcast to all partitions via PE
            c_sb = sp.tile([1, 1], F32, tag="c")
            nc.scalar.activation(c_sb[:], sums[0:1, 16:17], ACTF.Ln)
            # preload the Sigmoid table while ctx/GRU compute runs — the
            # post-CC2 gate activations then skip their table loads (tanh
            # is computed as 2*sigmoid(2x)-1 to stay on the same table).
            # Input depends on c_sb to pin this after the real Ln.
            sgwarm = sp.tile([1, 1], F32, tag="sgwarm")
            nc.scalar.activation(sgwarm[:], c_sb[:], ACTF.Sigmoid)

            # ctx xe half can run during the CC1 wait (dep: midc only)
            g_ps = pp.tile([128, 2], F32, tag="gps")
            for m in range(8):
                nc.tensor.matmul(g_ps[:, 0:1], midc[:, 128 * m:128 * (m + 1)],
                                 early[:, m:m + 1],
                                 start=(m == 0), stop=(m == 7))
            g_xe = sp.tile([128, 1], F32, tag="gxe")
            nc.vector.tensor_copy(g_xe[:], g_ps[:, 0:1])

            # gh = W_hh @ h0 depends only on inputs — run during the CC1
            # wait so the PE does it off the critical path (also keeps the
            # HAM clock-gate warm through the collective).
            gh_ps = pp.tile([128, 24], F32, tag="ghps")
            for c in range(24):
                nc.tensor.matmul(gh_ps[:, c:c + 1],
                                 midg[:, G_WHH + 128 * c:G_WHH + 128 * (c + 1)],
                                 midg[:, G_H0C:G_H0C + 1],
                                 start=True, stop=True)

            cb_ps = pp.tile([128, 4], F32, tag="apsA")
            nc.tensor.matmul(cb_ps[:, 0:1], ones_r[:], c_sb[:],
                             start=True, stop=True)
            cb_sb = sp.tile([128, 1], F32, tag="cb")
            nc.vector.tensor_copy(cb_sb[:], cb_ps[:, 0:1])

            # attn_applied = partial1_sum - c * colsum_sum   [128, 8]
            atmp = sp.tile([128, 8], F32, tag="atmp")
            nc.vector.tensor_scalar(atmp[:], sums[:, 8:16], cb_sb[:], None,
                                    mybir.AluOpType.mult)
            attnap = sp.tile([128, 8], F32, tag="attnap")
            nc.vector.tensor_sub(attnap[:], sums[:, 0:8], atmp[:])
            # split attnap hi/lo bf16
            ap_hi = sp.tile([128, 8], BF16, tag="aphi")
            nc.vector.tensor_copy(ap_hi[:], attnap[:])
            ap_hif = sp.tile([128, 8], F32, tag="aphif")
            nc.vector.tensor_copy(ap_hif[:], ap_hi[:])
            ap_lof = sp.tile([128, 8], F32, tag="aplof")
            nc.vector.tensor_sub(ap_lof[:], attnap[:], ap_hif[:])
            ap_lo = sp.tile([128, 8], BF16, tag="aplo")
            nc.vector.tensor_copy(ap_lo[:], ap_lof[:])

            # ---------------- context projection (attnap half) -----------
            nmm = 24
            k = 0
            for m in range(8):
                t = midc[:, 128 * (8 + m):128 * (9 + m)]
                nc.tensor.matmul(g_ps[:, 1:2], t, ap_hi[:, m:m + 1],
                                 start=(k == 0), stop=(k == nmm - 1))
                k += 1
                nc.tensor.matmul(g_ps[:, 1:2], t, ap_lo[:, m:m + 1],
                                 start=False, stop=(k == nmm - 1))
                k += 1
            for m in range(8):
                nc.tensor.matmul(g_ps[:, 1:2],
                                 midc[:, 2 * H + 128 * m:2 * H + 128 * (m + 1)],
                                 ap_hi[:, m:m + 1],
                                 start=False, stop=(k == nmm - 1))
                k += 1
            gpre = sp.tile([128, 1], F32, tag="gpre")
            nc.vector.tensor_add(gpre[:], g_ps[:, 1:2], g_xe[:])
            # relu on the vector engine: g = max(gpre + ctx_b, 0)
            g_sb = sp.tile([128, 1], F32, tag="g")
            nc.vector.tensor_scalar(g_sb[:], gpre[:],
                                    small[:, S_CB:S_CB + 1], 0.0,
                                    mybir.AluOpType.add,
                                    mybir.AluOpType.max)

            # gstat: [g_hi, g_lo, 0, g_hi, 0, 0, 0, h0c]
            gstat = sp.tile([128, 8], BF16, tag="gstat")
            nc.vector.memset(gstat[:, 2:3], 0.0)
            nc.vector.memset(gstat[:, 4:7], 0.0)
            nc.vector.tensor_copy(gstat[:, 7:8], midg[:, G_H0C:G_H0C + 1])
            nc.vector.tensor_copy(gstat[:, 0:1], g_sb[:])
            g_hif = sp.tile([128, 1], F32, tag="ghif")
            nc.vector.tensor_copy(g_hif[:], gstat[:, 0:1])
            g_lof = sp.tile([128, 1], F32, tag="glof")
            nc.vector.tensor_sub(g_lof[:], g_sb[:], g_hif[:])
            nc.vector.tensor_copy(gstat[:, 1:2], g_lof[:])
            nc.vector.tensor_copy(gstat[:, 3:4], gstat[:, 0:1])

            # ---------------- GRU gi matvecs ------------------------------
            # per out-tile c, psum cols (2c, 2c+1): first matmul streams
            # (g_hi, g_lo) against wih_hi, the second accumulates
            # (g_hi, 0) against wih_lo — 48 matmuls instead of 72.
            gi_ps = pp.tile([128, 48], F32, tag="gips")
            for c in range(24):
                nc.tensor.matmul(gi_ps[:, 2 * c:2 * c + 2],
                                 midg[:, G_WIH + 128 * c:G_WIH + 128 * (c + 1)],
                                 gstat[:, 0:2], start=True, stop=False)
                nc.tensor.matmul(gi_ps[:, 2 * c:2 * c + 2],
                                 midg[:, G_WIL + 128 * c:G_WIL + 128 * (c + 1)],
                                 gstat[:, 3:5], start=False, stop=True)
            giS = sp.tile([128, 48], F32, tag="giS")
            nc.vector.tensor_copy(giS[:], gi_ps[:])
            cc2_sb = sp.tile([128, 48], F32, tag="cc2sb")
            nc.vector.tensor_copy(cc2_sb[:, 24:48], gh_ps[:])
            nc.vector.tensor_add(cc2_sb[:, 0:24], giS[:, 0:48:2],
                                 giS[:, 1:48:2])

            # ---------------- collective 2: AllGather + local sum --------
            # gh is ready long before gi (it only needs h0/whh) — ship it
            # separately so its ~3us DMA completion latency hides inside
            # the CC1 wait instead of gating the CC2 doorbell.
            cc2_in = dp.tile([128, 48], F32, tag="cc2in")
            cc2_out = dp.tile([NCORES, 128, 48], F32, tag="cc2out",
                              addr_space="Shared")
            nc.scalar.dma_start(cc2_in[:, 24:48], cc2_sb[:, 24:48])
            nc.scalar.dma_start(cc2_in[:, 0:24], cc2_sb[:, 0:24])
            nc.gpsimd.collective_compute(AG, BYP, replica_groups=RG,
                                         ins=[cc2_in.opt()],
                                         outs=[cc2_out[:]])
            sums2 = sp.tile([128, 8 * 48], F32, tag="sums2")
            nc.scalar.dma_start(
                sums2[:].rearrange("p (r k) -> p r k", r=8),
                cc2_out[:].transpose([1, 0, 2]))
            u1 = sp.tile([128, 192], F32, tag="cc2t1")
            nc.vector.tensor_add(u1[:], sums2[:, 0:192], sums2[:, 192:384])
            u2 = sp.tile([128, 96], F32, tag="cc2t2")
            nc.vector.tensor_add(u2[:], u1[:, 0:96], u1[:, 96:192])
            u3 = sp.tile([128, 48], F32, tag="cc2t3")
            nc.vector.tensor_add(u3[:], u2[:, 0:48], u2[:, 48:96])
            # u3 cols: 0-23 gi, 24-47 gh (col-major over 3072)
            gi_b = sp.tile([128, 24], F32, tag="gib")
            nc.vector.tensor_add(gi_b[:], u3[:, 0:24], small[:, S_BI:S_BI + 24])
            gh_b = sp.tile([128, 24], F32, tag="ghb")
            nc.vector.tensor_add(gh_b[:], u3[:, 24:48], small[:, S_BH:S_BH + 24])

            # gates (PyTorch order r, z, n); r and z share one Sigmoid pass
            rzpre = sp.tile([128, 16], F32, tag="rzpre")
            nc.vector.tensor_add(rzpre[:], gi_b[:, 0:16], gh_b[:, 0:16])
            rz_sb = sp.tile([128, 16], F32, tag="rz")
            nc.scalar.activation(rz_sb[:], rzpre[:], ACTF.Sigmoid)
            npre = sp.tile([128, 8], F32, tag="npre")
            nc.vector.tensor_mul(npre[:], rz_sb[:, 0:8], gh_b[:, 16:24])
            nc.vector.tensor_add(npre[:], npre[:], gi_b[:, 16:24])
            # tanh(x) = 2*sigmoid(2x) - 1: stays on the loaded Sigmoid table
            nc.vector.tensor_scalar_mul(npre[:], npre[:], 2.0)
            nsig = sp.tile([128, 8], F32, tag="nsig")
            nc.scalar.activation(nsig[:], npre[:], ACTF.Sigmoid)
            n_sb = sp.tile([128, 8], F32, tag="n")
            nc.vector.tensor_scalar(n_sb[:], nsig[:], 2.0, -1.0,
                                    mybir.AluOpType.mult,
                                    mybir.AluOpType.add)
            # h_new = n + z * (h0 - n)
            hd = sp.tile([128, 8], F32, tag="hd")
            nc.vector.tensor_sub(hd[:], small[:, S_H0:S_H0 + 8], n_sb[:])
            nc.vector.tensor_mul(hd[:], hd[:], rz_sb[:, 8:16])
            hnew_b = sp.tile([128, 8], BF16, tag="hnewb")
            nc.vector.tensor_add(hnew_b[:], n_sb[:], hd[:])

            # heartbeats across the collective waits, keyed on out_W DMA
            # chunk arrivals (~3us apart) — keeps the PE HAM clock warm
            junk2 = pp.tile([1, 8], F32, tag="apsA")
            for k in range(OW_NCHUNK):
                col = min(OW_CHUNK * (k + 1), OW_N) - 1
                nc.tensor.matmul(junk2[:, k % 8:k % 8 + 1],
                                 early[:, 0:1], outw[:, col:col + 1],
                                 start=True, stop=True)
            # post-CC2 prewarm: a burst of junk matmuls keyed on the CC2
            # readback, running while the vector gate chain computes h_new
            jb = sp.tile([128, 1], BF16, tag="jb")
            nc.vector.tensor_copy(jb[:], sums2[:, 0:1])
            junk3 = pp.tile([16, 8], F32, tag="apsA")
            for k in range(20):
                nc.tensor.matmul(junk3[:, k % 8:k % 8 + 1],
                                 early[:, 0:16], jb[:],
                                 start=True, stop=True)
            junk2_rd = sp.tile([1, 8], F32, tag="junk2rd")
            nc.vector.tensor_copy(junk2_rd[:], junk2[:])
            junk3_rd = sp.tile([16, 8], F32, tag="junk3rd")
            nc.vector.tensor_copy(junk3_rd[:], junk3[:])

            # ---------------- output projection --------------------------
            logit_sb = sp.tile([128, VT], F32, tag="logit")
            t0c = 0
            while t0c < VT:
                nt = min(8, VT - t0c)
                o_ps = pp.tile([128, nt], F32, tag="gio", bufs=2)
                for t in range(t0c, t0c + nt):
                    for j in range(8):
                        nc.tensor.matmul(
                            o_ps[:, t - t0c:t - t0c + 1],
                            outw[:, (t * 8 + j) * 128:(t * 8 + j + 1) * 128],
                            hnew_b[:, j:j + 1],
                            start=(j == 0), stop=(j == 7))
                nc.vector.tensor_add(logit_sb[:, t0c:t0c + nt], o_ps[:],
                                     small[:, S_OB + t0c:S_OB + t0c + nt])
                nc.scalar.dma_start(out_d[:, t0c:t0c + nt],
                                    logit_sb[:, t0c:t0c + nt])
                t0c += nt

            if dbg:
                dbg_tiles = {
                    "dbg_a": a_sb, "dbg_exp": exp_sb, "dbg_pack": pack_sb,
                    "dbg_sums": sums, "dbg_cb": cb_sb, "dbg_attnap": attnap,
                    "dbg_g": g_sb, "dbg_u3": u3, "dbg_gib": gi_b,
                    "dbg_ghb": gh_b, "dbg_hnew": hnew_b,
                }
                for name, t in dbg_tiles.items():
                    shp = list(t[:].shape)
                    d = nc.dram_tensor(name, shp, t[:].dtype,
                                       kind="ExternalOutput")
                    nc.sync.dma_start(d[:], t[:])

    nc.compile()
    _CACHE[key] = nc
    return nc


def _col_major(v, ncols):
    # v [n] -> [128, ncols] with [p, c] = v[128 * c + p]
    return np.ascontiguousarray(v.reshape(ncols, 128).T)


def _pack_rows(a, nb):
    # a [nb*128, w] -> [128, nb*w] with [p, w*i + q] = a[128*i + p, q]
    w = a.shape[1]
    return np.ascontiguousarray(
        a.reshape(nb, 128, w).transpose(1, 0, 2).reshape(128, nb * w))


def _hi_lo(a):
    hi = a.astype(NPBF16)
    lo = (a - hi.astype(np.float32)).astype(NPBF16)
    return hi, lo


def _shard(inputs):
    x = np.asarray(inputs["x"]).reshape(-1)
    h0 = np.asarray(inputs["h"], dtype=np.float32).reshape(H)
    enc = np.asarray(inputs["encoder_outputs"], dtype=np.float32)
    emb = np.asarray(inputs["emb"])
    attn_W = np.asarray(inputs["attn_W"], dtype=np.float32)
    attn_b = np.asarray(inputs["attn_b"], dtype=np.float32)
    ctx_W = np.asarray(inputs["ctx_W"], dtype=np.float32)
    ctx_b = np.asarray(inputs["ctx_b"], dtype=np.float32)
    W_ih = np.asarray(inputs["W_ih"], dtype=np.float32)
    W_hh = np.asarray(inputs["W_hh"], dtype=np.float32)
    b_ih = np.asarray(inputs["b_ih"], dtype=np.float32)
    b_hh = np.asarray(inputs["b_hh"], dtype=np.float32)
    out_W = np.asarray(inputs["out_W"], dtype=np.float32)
    out_b = np.asarray(inputs["out_b"], dtype=np.float32)

    xe = np.asarray(emb[int(x[0])], dtype=np.float32)
    catin = np.concatenate([_col_major(xe, 8), _col_major(h0, 8)],
                           axis=1).astype(NPBF16)

    in_maps = []
    for k in range(NCORES):
        lsl = slice(LC * k, LC * (k + 1))
        hsl = slice(HC * k, HC * (k + 1))
        v0, v1 = VC * k, min(VC * (k + 1), V)
        # small f32
        small = np.zeros((128, S_N), dtype=np.float32)
        small[:, S_AB:S_AB + 4] = _col_major(attn_b[lsl], 4)
        small[:, S_CB] = ctx_b[hsl]
        small[:, S_H0:S_H0 + 8] = _col_major(h0, 8)
        small[:, S_BI:S_BI + 24] = _col_major(b_ih, 24)
        small[:, S_BH:S_BH + 24] = _col_major(b_hh, 24)
        ob = np.zeros(VPAD, dtype=np.float32)
        ob[:v1 - v0] = out_b[v0:v1]
        small[:, S_OB:S_OB + VT] = _col_major(ob, VT)
        # early bf16: catin | attn l-tile-major | enc
        A = np.ascontiguousarray(attn_W[lsl, :].T).astype(NPBF16)  # [2048,512]
        attn_pack = A.reshape(16, 128, 4, 128).transpose(
            1, 2, 0, 3).reshape(128, 16 * LC)
        enc_pack = _pack_rows(enc[lsl, :].astype(NPBF16), 4)
        early = np.concatenate(
            [catin, attn_pack, enc_pack], axis=1)
        # midc: ctx_hi(2048) | ctx_lo(1024)
        ctxT = np.ascontiguousarray(ctx_W[hsl, :].T)       # [2048, 128]
        ctx_hi = ctxT.astype(NPBF16)
        ctx_lo = (ctxT[H:] - ctx_hi[H:].astype(np.float32)).astype(NPBF16)
        midc = np.concatenate(
            [_pack_rows(ctx_hi, 16), _pack_rows(ctx_lo, 8)], axis=1)
        # midg: wih_hi | wih_lo | whh | h0c
        wihT = np.ascontiguousarray(W_ih[:, hsl].T)        # [128, 3072]
        wih_hi, wih_lo = _hi_lo(wihT)
        whhT = np.ascontiguousarray(W_hh[:, hsl].T).astype(NPBF16)
        midg = np.concatenate(
            [wih_hi, wih_lo, whhT,
             h0[hsl].reshape(128, 1).astype(NPBF16)], axis=1)
        # outw: vocab-tile-major [128, (t*8+j)*128 + q]
        owt = np.zeros((H, VPAD), dtype=NPBF16)
        owt[:, :v1 - v0] = out_W[v0:v1, :].T.astype(NPBF16)
        outw = owt.reshape(8, 128, VT, 128).transpose(
            1, 2, 0, 3).reshape(128, OW_N)

        in_maps.append({
            "small": np.ascontiguousarray(small),
            "early": np.ascontiguousarray(early),
            "midc": np.ascontiguousarray(midc),
            "midg": np.ascontiguousarray(midg),
            "outw": np.ascontiguousarray(outw),
        })
    return in_maps


def _gather(results):
    logits = np.empty(NCORES * VC, dtype=np.float32)
    for k in range(NCORES):
        chunk = np.asarray(results[k]["out"]).T.ravel()   # [VT*128]
        logits[VC * k:VC * (k + 1)] = chunk[:VC]
    return logits[:V].reshape(1, V)


def kernel(**inputs):
    nc = _build()
    in_maps = _shard(inputs)
    try:
        res = run_bass_kernel_spmd(nc, in_maps, core_ids=list(range(NCORES)))
    except Exception:
        # A dirty device state from a previous process occasionally fails
        # the first launch (NRT_EXEC_UNIT_UNRECOVERABLE); one retry clears.
        res = run_bass_kernel_spmd(nc, in_maps, core_ids=list(range(NCORES)))
    return _gather(res.results)


def kernel_traced(**inputs):
    """Like kernel() but profiles on HW; returns (output, exec_time_ns)."""
    nc = _build()
    in_maps = _shard(inputs)
    res = run_bass_kernel_spmd(nc, in_maps, core_ids=list(range(NCORES)),
                               trace=True)
    return _gather(res.results), res.exec_time_ns


def kernel_debug(**inputs):
    """Run the debug build; returns per-core dicts of all outputs."""
    nc = _build(dbg=True)
    in_maps = _shard(inputs)
    res = run_bass_kernel_spmd(nc, in_maps, core_ids=list(range(NCORES)))
    return res.results
